# revision 22
# baseline (speedup 1.0000x reference)
"""Trainium2 Bass kernel for the adaptive-span Mamba block (moe_routing).

V2: host packs routing (~23% active (position,group) pairs), sparse bilinear
blend matrices S (windows materialize as PE matmuls: win = bands^T @ S over
per-token-tile pixel slots), scan reset zones, and pair->position scatter
matrices E, per core (SPMD: one program, per-core data; program shape only
depends on bucketed capacities). Device: RMSNorm stats via ones-matmul + row
rsqrt, Win/Wout matmuls (PE bf16, channel-major, 512-token PSUM tiles), Mamba
recurrence via native DVE tensor_tensor_scan reading u straight from PSUM
with cross-tile carry, depth-2 evaluated only to each window's center token,
aggregators as PE matmuls with a scatter matmul per group.

Only tokens 0..center feed the outputs (causal scan, only center read), so
sequences truncate to the (half+1) x s leading rectangle of each window.
"""

import os
import sys
import math
import numpy as np

for _p in ("/opt/trn_rl_repo",):
    if _p not in sys.path:
        sys.path.insert(0, _p)

import concourse.bass as bass
import concourse.bacc as bacc
import concourse.tile as tile
import concourse.mybir as mybir
from concourse.bass_utils import run_bass_kernel_spmd

BF16 = mybir.dt.bfloat16
F32 = mybir.dt.float32
NPBF16 = mybir.dt.np(mybir.dt.bfloat16)
ALU = mybir.AluOpType
ACTF = mybir.ActivationFunctionType

SPAN_GROUPS = (5, 7, 9, 11, 15)
TOL = 1.5
DEPTH = 2
B, C, DG, H, W = 1, 256, 64, 16, 16
G = len(SPAN_GROUPS)
N_CORES = 8
P_CORE = 64
PAD = 7
IMG = H + PAD + 8    # 31
NPIX = IMG * IMG     # 961
EPS = 1e-6
TOKTILE = 512
USE_SILU = os.environ.get("KERNEL_NO_SILU", "") != "1"

HALF = [s // 2 for s in SPAN_GROUPS]
PSZ_R = [h + 2 for h in HALF]
PSZ_C = [s + 1 for s in SPAN_GROUPS]
T_R = [h + 1 for h in HALF]
T_G = [(h + 1) * s for h, s in zip(HALF, SPAN_GROUPS)]
CENTER = [h * s + h for h, s in zip(HALF, SPAN_GROUPS)]


# --------------------------------------------------------------------------
# host-side routing / packing
# --------------------------------------------------------------------------

def _routing(inputs):
    pairs = [[[] for _ in range(G)] for _ in range(N_CORES)]
    for s_id in range(2):
        flow = np.asarray(inputs[f"flow_map_{s_id}"])[0]
        sx = np.asarray(inputs[f"adaptive_spans_x_{s_id}"])[0]
        sy = np.asarray(inputs[f"adaptive_spans_y_{s_id}"])[0]
        yy, xx = np.meshgrid(np.arange(H, dtype=np.float32),
                             np.arange(W, dtype=np.float32), indexing="ij")
        cy = np.clip(yy + flow[..., 1], 0, H - 1).astype(np.float32)
        cx = np.clip(xx + flow[..., 0], 0, W - 1).astype(np.float32)
        y0i = np.floor(cy).astype(np.int64)
        x0i = np.floor(cx).astype(np.int64)
        fy = (cy - y0i).astype(np.float32)
        fx = (cx - x0i).astype(np.float32)
        avg = ((sx + sy) * 0.5).astype(np.float32)
        for n in range(H * W):
            yi, xi = divmod(n, W)
            p_glob = s_id * H * W + n
            core, local = p_glob % N_CORES, p_glob // N_CORES
            for g, s in enumerate(SPAN_GROUPS):
                if abs(float(avg[yi, xi]) - s) < TOL:
                    pairs[core][g].append(
                        (local, s_id, int(y0i[yi, xi]), int(x0i[yi, xi]),
                         float(fy[yi, xi]), float(fx[yi, xi])))
    for core in range(N_CORES):
        for g in range(G):
            pairs[core][g].sort(key=lambda t: (t[1], t[2]))
    return pairs


def _caps_from_pairs(pairs):
    caps = []
    for g in range(G):
        m = max(len(pairs[c][g]) for c in range(N_CORES))
        caps.append(max(2, int(math.ceil(m / 2.0)) * 2))
    return tuple(caps)


def _tile_pixel_sets(pairs_g, cap, g):
    """Per token-tile: sorted list of (stream, pixel) the tile's pairs use."""
    tg, half, pr, pc = T_G[g], HALF[g], PSZ_R[g], PSZ_C[g]
    X = cap * tg
    ntt = (X + TOKTILE - 1) // TOKTILE
    sets = [set() for _ in range(ntt)]
    for i, (local, s_id, y0, x0, fy, fx) in enumerate(pairs_g):
        r0, c0 = y0 - half + PAD, x0 - half + PAD
        pix = [(s_id, (r0 + r) * IMG + c0 + c)
               for r in range(pr) for c in range(pc)]
        for tt in range(i * tg // TOKTILE, ((i + 1) * tg - 1) // TOKTILE + 1):
            if tt < ntt:
                sets[tt].update(pix)
    return [sorted(ps) for ps in sets]


def _nslots(pairs, caps):
    """Per-group, per-token-tile slot counts (max across cores)."""
    ns = []
    for g in range(G):
        X = caps[g] * T_G[g]
        ntt = (X + TOKTILE - 1) // TOKTILE
        per_tt = [1] * ntt
        for core in range(N_CORES):
            for tt, ps in enumerate(
                    _tile_pixel_sets(pairs[core][g], caps[g], g)):
                per_tt[tt] = max(per_tt[tt], (len(ps) + 127) // 128)
        ns.append(tuple(per_tt))
    return tuple(ns)


def _pack_core(inputs, pairs_core, caps, nslots):
    imgs = []
    for s_id in range(2):
        feat = np.asarray(inputs[f"feat_match_{s_id}"])[0]
        img = np.zeros((C, IMG, IMG), np.float32)
        img[:, PAD:PAD + H, PAD:PAD + W] = feat
        imgs.append(img.reshape(C, NPIX))
    out = {}
    for g, s in enumerate(SPAN_GROUPS):
        cap, half, tg = caps[g], HALF[g], T_G[g]
        nsl_tt = nslots[g]
        X = cap * tg
        ntt = (X + TOKTILE - 1) // TOKTILE
        soff = np.cumsum([0] + list(nsl_tt))  # slot offsets per tile
        tot_sl = int(soff[-1])
        psets = _tile_pixel_sets(pairs_core[g], cap, g)
        pidx = [{p: j for j, p in enumerate(ps)} for ps in psets]
        bands = np.zeros((128, tot_sl, 2, 128), np.float32)
        spack = np.zeros((128, tot_sl, TOKTILE), np.float32)
        zones = np.ones((128, cap, tg), np.float32)
        zones[:, :, 0] = 0.0
        E = np.zeros((cap + (1 if g == 0 else 0), P_CORE), np.float32)
        for tt in range(ntt):
            for j, (s_id, pix) in enumerate(psets[tt]):
                sl, row = divmod(j, 128)
                bands[row, soff[tt] + sl, :, :] = \
                    imgs[s_id][:, pix].reshape(2, 128)
        for i, (local, s_id, y0, x0, fy, fx) in enumerate(pairs_core[g]):
            E[i, local] = 1.0
            r0, c0 = y0 - half + PAD, x0 - half + PAD
            ws = ((1 - fy) * (1 - fx), (1 - fy) * fx,
                  fy * (1 - fx), fy * fx)
            dydx = ((0, 0), (0, 1), (1, 0), (1, 1))
            for l in range(tg):
                iy, ix = divmod(l, s)
                col = i * tg + l
                tt, colin = divmod(col, TOKTILE)
                for w, (dy, dx) in zip(ws, dydx):
                    pix = (s_id, (r0 + iy + dy) * IMG + c0 + ix + dx)
                    sl, row = divmod(pidx[tt][pix], 128)
                    spack[row, soff[tt] + sl, colin] += w
        if g == 0:
            E[cap, :] = 1.0
        out[f"bands_{g}"] = np.ascontiguousarray(
            bands.reshape(128, -1)).astype(NPBF16)
        out[f"spack_{g}"] = np.ascontiguousarray(
            spack.reshape(128, -1)).astype(NPBF16)
        out[f"zones_{g}"] = np.ascontiguousarray(
            zones.reshape(128, X)).astype(NPBF16)
        out[f"emat_{g}"] = np.ascontiguousarray(E).astype(NPBF16)
    return out


def _pack_weights(inputs):
    Win = np.asarray(inputs["Win"], np.float32)
    alog = np.asarray(inputs["alog"], np.float32)
    Wout = np.asarray(inputs["Wout"], np.float32)
    Wgeom = np.asarray(inputs["Wgeom"], np.float32)
    gamma = np.asarray(inputs["gamma"], np.float32)
    Wagg = np.asarray(inputs["Wagg"], np.float32)
    bagg = np.asarray(inputs["bagg"], np.float32)
    Wagg_g = np.asarray(inputs["Wagg_g"], np.float32)
    bagg_g = np.asarray(inputs["bagg_g"], np.float32)

    Winp = Win * gamma[..., None]
    wi = Winp.reshape(G, 2, 2, 128, 2 * C).transpose(3, 0, 1, 2, 4)
    wo = Wout.reshape(G, 2, 2, 128, C).transpose(3, 0, 1, 2, 4)
    wg = Wgeom[:, 1].reshape(G, 2, 128, DG).transpose(2, 0, 1, 3)
    wa = Wagg.reshape(G, 2, 128, C).transpose(2, 0, 1, 3)
    wag = Wagg_g.reshape(G, DG, DG).transpose(1, 0, 2)
    a = 1.0 / (1.0 + np.exp(-alog))
    av = a.reshape(G, 2, 2, 128).transpose(3, 0, 1, 2)
    return {
        "w_in": np.ascontiguousarray(wi.reshape(128, -1)).astype(NPBF16),
        "w_out": np.ascontiguousarray(wo.reshape(128, -1)).astype(NPBF16),
        "w_geom": np.ascontiguousarray(wg.reshape(128, -1)).astype(NPBF16),
        "w_agg": np.ascontiguousarray(wa.reshape(128, -1)).astype(NPBF16),
        "w_agg_g": np.ascontiguousarray(wag.reshape(DG, -1)).astype(NPBF16),
        "avals": np.ascontiguousarray(av.reshape(128, -1)),
        "bagg_row": bagg.reshape(1, C).astype(NPBF16),
        "bagg_g_row": bagg_g.reshape(1, DG).astype(NPBF16),
        "eye": np.eye(128, dtype=NPBF16),
    }


# --------------------------------------------------------------------------
# device program
# --------------------------------------------------------------------------

def build_program(caps, nslots):
    nc = bacc.Bacc("TRN2", target_bir_lowering=False, debug=False,
                   num_devices=N_CORES)
    din = {}

    def inp(name, shape, dt=BF16):
        din[name] = nc.dram_tensor(name, shape, dt,
                                   kind="ExternalInput").ap()

    inp("w_in", [128, G * 2 * 2 * 2 * C])
    inp("w_out", [128, G * 2 * 2 * C])
    inp("w_geom", [128, G * 2 * DG])
    inp("w_agg", [128, G * 2 * C])
    inp("w_agg_g", [DG, G * DG])
    inp("avals", [128, G * 2 * 2], F32)
    inp("bagg_row", [1, C])
    inp("bagg_g_row", [1, DG])
    inp("eye", [128, 128])
    for g in range(G):
        cap = caps[g]
        X = cap * T_G[g]
        tot_sl = sum(nslots[g])
        inp(f"bands_{g}", [128, tot_sl * 2 * 128])
        inp(f"spack_{g}", [128, tot_sl * TOKTILE])
        inp(f"zones_{g}", [128, X])
        inp(f"emat_{g}", [cap + (1 if g == 0 else 0), P_CORE])

    m_out = nc.dram_tensor("m_slice", [P_CORE, C], F32,
                           kind="ExternalOutput").ap()
    g_out = nc.dram_tensor("g_slice", [P_CORE, DG], F32,
                           kind="ExternalOutput").ap()

    with tile.TileContext(nc) as tc:
        _emit(tc, nc, din, m_out, g_out, caps, nslots)
    nc.compile()
    return nc


def _emit(tc, nc, din, m_out, g_out, caps, nslots):
    from contextlib import ExitStack
    ctx = ExitStack()
    cpool = ctx.enter_context(tc.tile_pool(name="const", bufs=1))
    gpool = ctx.enter_context(tc.tile_pool(name="grp", bufs=1))
    ppool = ctx.enter_context(tc.tile_pool(name="stage", bufs=1))
    mm_ps = ctx.enter_context(tc.tile_pool(name="mmps", bufs=1, space="PSUM"))
    row_ps = ctx.enter_context(tc.tile_pool(name="rowps", bufs=1, space="PSUM"))
    sm_ps = ctx.enter_context(tc.tile_pool(name="smps", bufs=1, space="PSUM"))

    w_in = cpool.tile([128, G * 2 * 2 * 2 * C], BF16)
    nc.sync.dma_start(out=w_in[:], in_=din["w_in"][:])
    w_outw = cpool.tile([128, G * 2 * 2 * C], BF16)
    nc.sync.dma_start(out=w_outw[:], in_=din["w_out"][:])
    w_geom = cpool.tile([128, G * 2 * DG], BF16)
    nc.sync.dma_start(out=w_geom[:], in_=din["w_geom"][:])
    w_agg = cpool.tile([128, G * 2 * C], BF16)
    nc.sync.dma_start(out=w_agg[:], in_=din["w_agg"][:])
    w_agg_g = cpool.tile([DG, G * DG], BF16)
    nc.sync.dma_start(out=w_agg_g[:], in_=din["w_agg_g"][:])
    avals = cpool.tile([128, G * 2 * 2], F32)
    nc.sync.dma_start(out=avals[:], in_=din["avals"][:])
    eye = cpool.tile([128, 128], BF16)
    nc.sync.dma_start(out=eye[:], in_=din["eye"][:])
    ones_col = cpool.tile([128, 1], BF16)
    nc.vector.memset(ones_col[:], 1.0)
    eps_tile = cpool.tile([128, 1], F32)
    nc.vector.memset(eps_tile[:], EPS)
    ones_row = cpool.tile([1, 128], BF16)
    nc.vector.memset(ones_row[:], 1.0)
    rs_ones_wide = cpool.tile([1, TOKTILE], BF16)
    nc.vector.memset(rs_ones_wide[:], 1.0)

    def win_lhsT(g, d, k, m):
        base = ((g * 2 + d) * 2 + k) * (2 * C)
        return w_in[:, base + m * 128: base + (m + 1) * 128]

    def wout_lhsT(g, d, k, m):
        base = ((g * 2 + d) * 2 + k) * C
        return w_outw[:, base + m * 128: base + (m + 1) * 128]

    def wgeom_lhsT(g, k):
        return w_geom[:, (g * 2 + k) * DG: (g * 2 + k + 1) * DG]

    def wagg_lhsT(g, k, m):
        base = (g * 2 + k) * C
        return w_agg[:, base + m * 128: base + (m + 1) * 128]

    m_acc = cpool.tile([P_CORE, C], F32)
    g_acc = cpool.tile([P_CORE, DG], F32)

    # warm up the PE clock (HAM): ~25 back-to-back dummy matmuls
    warm_sb = cpool.tile([128, 512], BF16)
    nc.vector.memset(warm_sb[:], 0.0)
    for wi in range(25):
        wps = mm_ps.tile([128, TOKTILE], F32, tag="w0")
        nc.tensor.matmul(wps[:], warm_sb[:, :128], warm_sb[:],
                         start=True, stop=True)

    for g in range(G):
        s = SPAN_GROUPS[g]
        cap, half = caps[g], HALF[g]
        tg, cen, nsl_tt = T_G[g], CENTER[g], nslots[g]
        X = cap * tg
        ntt = (X + TOKTILE - 1) // TOKTILE
        soff = [0]
        for v in nsl_tt:
            soff.append(soff[-1] + v)
        tot_sl = soff[-1]
        capg = cap + (1 if g == 0 else 0)

        bands = ppool.tile([128, tot_sl * 2 * 128], BF16, tag="bands")
        nc.sync.dma_start(out=bands[:], in_=din[f"bands_{g}"][:])
        spack = ppool.tile([128, tot_sl * TOKTILE], BF16, tag="spack")
        nc.sync.dma_start(out=spack[:], in_=din[f"spack_{g}"][:])
        zones = ppool.tile([128, X], BF16, tag="zones")
        nc.sync.dma_start(out=zones[:], in_=din[f"zones_{g}"][:])
        emat = ppool.tile([capg, P_CORE], BF16, tag="emat")
        nc.sync.dma_start(out=emat[:], in_=din[f"emat_{g}"][:])

        def bands_lhsT(tt, sl, m):
            base = ((soff[tt] + sl) * 2 + m) * 128
            return bands[:, base: base + 128]

        def spack_rhs(tt, sl, tl):
            base = (soff[tt] + sl) * TOKTILE
            return spack[:, base: base + tl]

        x0 = gpool.tile([128, 2, X], BF16, tag="x0", bufs=2)
        x1 = gpool.tile([128, 2, X], BF16, tag="x1")
        u_t = gpool.tile([128, 2, X], BF16, tag="uz")
        z_t = u_t  # sigmoid scratch in d1; u in d2 (disjoint lifetimes)
        h_t = gpool.tile([128, 2, X], BF16, tag="h")
        y_t = gpool.tile([128, 2, X], BF16, tag="y")
        x2 = gpool.tile([128, 2, X], BF16, tag="x2", bufs=2)
        xn = gpool.tile([128, 2, X], BF16, tag="xn")
        a_t = gpool.tile([128, 2, X], F32, tag="a_t")
        s_row = gpool.tile([1, X], F32, tag="s_row")
        r_row = gpool.tile([1, X], F32, tag="r_row")
        rs_row = gpool.tile([1, X], BF16, tag="rs_row")
        rs_sb = gpool.tile([128, X], BF16, tag="rs_sb")

        # ---- windows via PE: win = bands^T @ S; extract x0 + squares
        for tt in range(ntt):
            t0 = tt * TOKTILE
            tl = min(TOKTILE, X - t0)
            for m in range(2):
                wp = mm_ps.tile([128, TOKTILE], F32, tag=("w0", "w1")[m])
                for sl in range(nsl_tt[tt]):
                    nc.tensor.matmul(wp[:, :tl], bands_lhsT(tt, sl, m),
                                     spack_rhs(tt, sl, tl),
                                     start=(sl == 0),
                                     stop=(sl == nsl_tt[tt] - 1))
                nc.scalar.activation(out=x0[:, m, t0:t0 + tl],
                                     in_=wp[:, :tl], func=ACTF.Copy)
                nc.scalar.activation(out=x2[:, m, t0:t0 + tl],
                                     in_=wp[:, :tl], func=ACTF.Square)

        for d in range(DEPTH):
            if d == 1:
                for ch in range(2):
                    nc.scalar.activation(out=x2[:, ch], in_=x1[:, ch],
                                         func=ACTF.Square)
            # rs row: 1/sqrt(mean(x^2) + eps)
            for t0 in range(0, X, TOKTILE):
                tl = min(TOKTILE, X - t0)
                s_ps = row_ps.tile([1, TOKTILE], F32, tag="row")
                for ch in range(2):
                    nc.tensor.matmul(s_ps[:, :tl], ones_col[:],
                                     x2[:, ch, t0:t0 + tl],
                                     start=(ch == 0), stop=(ch == 1))
                nc.scalar.activation(out=s_row[:, t0:t0 + tl],
                                     in_=s_ps[:, :tl], func=ACTF.Copy)
            # rs = exp(-0.5*ln(sum/C + eps)); single Ln/Exp per depth to
            # minimize ACT table switches
            nc.scalar.activation(out=r_row[:], in_=s_row[:], func=ACTF.Ln,
                                 scale=1.0 / C, bias=eps_tile[:1, :])
            nc.scalar.activation(out=rs_row[:], in_=r_row[:],
                                 func=ACTF.Exp, scale=-0.5)
            for t0 in range(0, X, TOKTILE):
                tl = min(TOKTILE, X - t0)
                bc_ps = row_ps.tile([128, TOKTILE], F32, tag="row")
                nc.tensor.matmul(bc_ps[:, :tl], ones_row[:],
                                 rs_row[:, t0:t0 + tl], start=True, stop=True)
                nc.scalar.activation(out=rs_sb[:, t0:t0 + tl],
                                     in_=bc_ps[:, :tl], func=ACTF.Copy)

            # decay tile (fp32): zones * a  (DVE TS, per token-tile)
            for ch in range(2):
                av = avals[:, (g * 2 + d) * 2 + ch: (g * 2 + d) * 2 + ch + 1]
                for t0 in range(0, X, TOKTILE):
                    tl = min(TOKTILE, X - t0)
                    nc.vector.tensor_scalar_mul(
                        out=a_t[:, ch, t0:t0 + tl],
                        in0=zones[:, t0:t0 + tl], scalar1=av)

            if d == 0:
                for tt0 in range(ntt):
                    t0 = tt0 * TOKTILE
                    tl = min(TOKTILE, X - t0)
                    for ch in range(2):
                        nc.vector.tensor_tensor(
                            out=xn[:, ch, t0:t0 + tl],
                            in0=x0[:, ch, t0:t0 + tl],
                            in1=rs_sb[:, t0:t0 + tl], op=ALU.mult)
                for tt in range(ntt):
                    t0 = tt * TOKTILE
                    tl = min(TOKTILE, X - t0)
                    uzps = []
                    for m in range(4):
                        ps = mm_ps.tile([128, TOKTILE], F32,
                                        tag=("u0", "u1", "z0", "z1")[m])
                        for k in range(2):
                            nc.tensor.matmul(ps[:, :tl], win_lhsT(g, d, k, m),
                                             xn[:, k, t0:t0 + tl],
                                             start=(k == 0), stop=(k == 1))
                        uzps.append(ps)
                    for ch in range(2):
                        init = 0.0 if tt == 0 else h_t[:, ch, t0 - 1:t0]
                        nc.vector.tensor_tensor_scan(
                            out=h_t[:, ch, t0:t0 + tl],
                            data0=a_t[:, ch, t0:t0 + tl],
                            data1=uzps[ch][:, :tl],
                            initial=init, op0=ALU.mult, op1=ALU.add)
                        if USE_SILU:
                            nc.scalar.activation(
                                out=z_t[:, ch, t0:t0 + tl],
                                in_=uzps[2 + ch][:, :tl], func=ACTF.Silu)
                            nc.vector.tensor_tensor(
                                out=y_t[:, ch, t0:t0 + tl],
                                in0=h_t[:, ch, t0:t0 + tl],
                                in1=z_t[:, ch, t0:t0 + tl], op=ALU.mult)
                        else:
                            nc.scalar.activation(
                                out=z_t[:, ch, t0:t0 + tl],
                                in_=uzps[2 + ch][:, :tl], func=ACTF.Sigmoid)
                            nc.vector.tensor_tensor(
                                out=y_t[:, ch, t0:t0 + tl],
                                in0=h_t[:, ch, t0:t0 + tl],
                                in1=z_t[:, ch, t0:t0 + tl], op=ALU.mult)
                            nc.vector.tensor_tensor(
                                out=y_t[:, ch, t0:t0 + tl],
                                in0=y_t[:, ch, t0:t0 + tl],
                                in1=uzps[2 + ch][:, :tl], op=ALU.mult)
                for tt in range(ntt):
                    t0 = tt * TOKTILE
                    tl = min(TOKTILE, X - t0)
                    for m in range(2):
                        ps = mm_ps.tile([128, TOKTILE], F32,
                                        tag=("u0", "u1")[m])
                        for k in range(2):
                            nc.tensor.matmul(ps[:, :tl], wout_lhsT(g, d, k, m),
                                             y_t[:, k, t0:t0 + tl],
                                             start=(k == 0), stop=(k == 1))
                        nc.vector.tensor_tensor(
                            out=x1[:, m, t0:t0 + tl], in0=ps[:, :tl],
                            in1=x0[:, m, t0:t0 + tl], op=ALU.add)
            else:
                for tt in range(ntt):
                    t0 = tt * TOKTILE
                    tl = min(TOKTILE, X - t0)
                    for ch in range(2):
                        nc.vector.tensor_tensor(
                            out=xn[:, ch, t0:t0 + tl],
                            in0=x1[:, ch, t0:t0 + tl],
                            in1=rs_sb[:, t0:t0 + tl], op=ALU.mult)
                    for m in range(2):
                        ps = mm_ps.tile([128, TOKTILE], F32,
                                        tag=("u0", "u1")[m])
                        for k in range(2):
                            nc.tensor.matmul(ps[:, :tl], win_lhsT(g, d, k, m),
                                             xn[:, k, t0:t0 + tl],
                                             start=(k == 0), stop=(k == 1))
                        init = 0.0 if tt == 0 else h_t[:, m, t0 - 1:t0]
                        nc.vector.tensor_tensor_scan(
                            out=h_t[:, m, t0:t0 + tl],
                            data0=a_t[:, m, t0:t0 + tl],
                            data1=ps[:, :tl],
                            initial=init, op0=ALU.mult, op1=ALU.add)

        # ---- depth-2 center-only tail
        h_c = [h_t[:, ch].rearrange("p (a b) -> p a b", a=cap)[:, :, cen]
               for ch in range(2)]
        x1_c = [x1[:, ch].rearrange("p (a b) -> p a b", a=cap)[:, :, cen]
                for ch in range(2)]
        xn_c = [xn[:, ch].rearrange("p (a b) -> p a b", a=cap)[:, :, cen]
                for ch in range(2)]

        z_c = gpool.tile([128, 2, cap], BF16, tag="z_c")
        y_c = gpool.tile([128, 2, cap], BF16, tag="y_c")
        mc = gpool.tile([128, 2, cap], BF16, tag="mc")
        gc = gpool.tile([DG, cap], BF16, tag="gc")

        for m in range(2):
            ps = sm_ps.tile([128, cap], F32, tag="smps")
            for k in range(2):
                nc.tensor.matmul(ps[:], win_lhsT(g, 1, k, 2 + m),
                                 xn_c[k], start=(k == 0), stop=(k == 1))
            nc.scalar.activation(out=z_c[:, m], in_=ps[:], func=ACTF.Copy)
        for ch in range(2):
            if USE_SILU:
                nc.scalar.activation(out=z_c[:, ch], in_=z_c[:, ch],
                                     func=ACTF.Silu)
                nc.vector.tensor_tensor(out=y_c[:, ch], in0=h_c[ch],
                                        in1=z_c[:, ch], op=ALU.mult)
            else:
                nc.vector.tensor_tensor(out=y_c[:, ch], in0=h_c[ch],
                                        in1=z_c[:, ch], op=ALU.mult)
                nc.scalar.activation(out=z_c[:, ch], in_=z_c[:, ch],
                                     func=ACTF.Sigmoid)
                nc.vector.tensor_tensor(out=y_c[:, ch], in0=y_c[:, ch],
                                        in1=z_c[:, ch], op=ALU.mult)
        for m in range(2):
            ps = sm_ps.tile([128, cap], F32, tag="smps")
            for k in range(2):
                nc.tensor.matmul(ps[:], wout_lhsT(g, 1, k, m),
                                 y_c[:, k], start=(k == 0), stop=(k == 1))
            nc.vector.tensor_tensor(out=mc[:, m], in0=ps[:], in1=x1_c[m],
                                    op=ALU.add)
        ps = row_ps.tile([DG, cap], F32, tag="row")
        for k in range(2):
            nc.tensor.matmul(ps[:], wgeom_lhsT(g, k), y_c[:, k],
                             start=(k == 0), stop=(k == 1))
        nc.scalar.activation(out=gc[:], in_=ps[:], func=ACTF.Copy)

        # ---- aggregation
        pcon = gpool.tile([128, 2, cap], BF16, tag="pcon")
        for m in range(2):
            ps = sm_ps.tile([128, cap], F32, tag="smps")
            for k in range(2):
                nc.tensor.matmul(ps[:], wagg_lhsT(g, k, m), mc[:, k],
                                 start=(k == 0), stop=(k == 1))
            nc.scalar.activation(out=pcon[:, m], in_=ps[:], func=ACTF.Copy)
        gcon = gpool.tile([DG, cap], BF16, tag="gcon")
        ps = row_ps.tile([DG, cap], F32, tag="row")
        nc.tensor.matmul(ps[:], w_agg_g[:, g * DG:(g + 1) * DG], gc[:],
                         start=True, stop=True)
        nc.scalar.activation(out=gcon[:], in_=ps[:], func=ACTF.Copy)

        pairT = gpool.tile([capg, C], BF16, tag="pairT")
        gpairT = gpool.tile([capg, DG], BF16, tag="gpairT")
        for m in range(2):
            tp = sm_ps.tile([cap, 128], BF16, tag="smps")
            nc.tensor.transpose(tp[:], pcon[:, m], eye[:])
            nc.vector.tensor_copy(pairT[:cap, m * 128:(m + 1) * 128], tp[:])
        tpg = row_ps.tile([cap, DG], BF16, tag="row")
        nc.tensor.transpose(tpg[:], gcon[:], eye[:DG, :DG])
        nc.vector.tensor_copy(gpairT[:cap, :], tpg[:])
        if g == 0:
            nc.sync.dma_start(out=pairT[cap:cap + 1, :],
                              in_=din["bagg_row"][:])
            nc.sync.dma_start(out=gpairT[cap:cap + 1, :],
                              in_=din["bagg_g_row"][:])

        mg_ps = sm_ps.tile([P_CORE, C], F32, tag="smps")
        nc.tensor.matmul(mg_ps[:], emat[:], pairT[:], start=True, stop=True)
        gg_ps = row_ps.tile([P_CORE, DG], F32, tag="row")
        nc.tensor.matmul(gg_ps[:], emat[:], gpairT[:], start=True, stop=True)
        if g == 0:
            nc.scalar.activation(out=m_acc[:], in_=mg_ps[:], func=ACTF.Copy)
            nc.scalar.activation(out=g_acc[:], in_=gg_ps[:], func=ACTF.Copy)
        else:
            nc.vector.tensor_tensor(out=m_acc[:], in0=m_acc[:], in1=mg_ps[:],
                                    op=ALU.add)
            nc.vector.tensor_tensor(out=g_acc[:], in0=g_acc[:], in1=gg_ps[:],
                                    op=ALU.add)

    nc.sync.dma_start(out=m_out[:], in_=m_acc[:])
    nc.sync.dma_start(out=g_out[:], in_=g_acc[:])
    ctx.close()


# --------------------------------------------------------------------------
# entry point
# --------------------------------------------------------------------------

_PROGRAM_CACHE = {}


def _get_program(caps, nslots):
    key = (caps, nslots, USE_SILU)
    if key not in _PROGRAM_CACHE:
        _PROGRAM_CACHE[key] = build_program(caps, nslots)
    return _PROGRAM_CACHE[key]


def make_in_maps(inputs):
    pairs = _routing(inputs)
    caps = _caps_from_pairs(pairs)
    nslots = _nslots(pairs, caps)
    wmaps = _pack_weights(inputs)
    in_maps = []
    for core in range(N_CORES):
        m = dict(wmaps)
        m.update(_pack_core(inputs, pairs[core], caps, nslots))
        in_maps.append(m)
    return caps, nslots, in_maps


def _assemble(results):
    m_full = np.zeros((512, C), np.float32)
    g_full = np.zeros((512, DG), np.float32)
    for c in range(N_CORES):
        m_full[c::N_CORES] = results[c]["m_slice"]
        g_full[c::N_CORES] = results[c]["g_slice"]
    outs = []
    for s_id in range(2):
        m = m_full[s_id * 256:(s_id + 1) * 256]
        outs.append(np.ascontiguousarray(
            m.reshape(H, W, C).transpose(2, 0, 1)[None]).astype(np.float32))
    for s_id in range(2):
        gsl = g_full[s_id * 256:(s_id + 1) * 256]
        outs.append(np.ascontiguousarray(
            gsl.reshape(H, W, DG).transpose(2, 0, 1)[None]).astype(np.float32))
    return tuple(outs)


def kernel(**inputs):
    caps, nslots, in_maps = make_in_maps(inputs)
    nc = _get_program(caps, nslots)
    res = run_bass_kernel_spmd(nc, in_maps, list(range(N_CORES)))
    return _assemble(res.results)


if __name__ == "__main__":
    nc = build_program((8, 20, 32, 32, 12), (2, 3, 3, 3, 4))
    print("built+compiled ok")


# revision 23
# speedup vs baseline: 1.1989x; 1.1989x over previous
"""Trainium2 Bass kernel for the adaptive-span Mamba block (moe_routing).

V2: host packs routing (~23% active (position,group) pairs), sparse bilinear
blend matrices S (windows materialize as PE matmuls: win = bands^T @ S over
per-token-tile pixel slots), scan reset zones, and pair->position scatter
matrices E, per core (SPMD: one program, per-core data; program shape only
depends on bucketed capacities). Device: RMSNorm stats via ones-matmul + row
rsqrt, Win/Wout matmuls (PE bf16, channel-major, 512-token PSUM tiles), Mamba
recurrence via native DVE tensor_tensor_scan reading u straight from PSUM
with cross-tile carry, depth-2 evaluated only to each window's center token,
aggregators as PE matmuls with a scatter matmul per group.

Only tokens 0..center feed the outputs (causal scan, only center read), so
sequences truncate to the (half+1) x s leading rectangle of each window.
"""

import os
import sys
import math
import numpy as np

for _p in ("/opt/trn_rl_repo",):
    if _p not in sys.path:
        sys.path.insert(0, _p)

import concourse.bass as bass
import concourse.bacc as bacc
import concourse.tile as tile
import concourse.mybir as mybir
from concourse.bass_utils import run_bass_kernel_spmd

BF16 = mybir.dt.bfloat16
F32 = mybir.dt.float32
NPBF16 = mybir.dt.np(mybir.dt.bfloat16)
ALU = mybir.AluOpType
ACTF = mybir.ActivationFunctionType

SPAN_GROUPS = (5, 7, 9, 11, 15)
TOL = 1.5
DEPTH = 2
B, C, DG, H, W = 1, 256, 64, 16, 16
G = len(SPAN_GROUPS)
N_CORES = 8
P_CORE = 64
PAD = 7
IMG = H + PAD + 8    # 31
NPIX = IMG * IMG     # 961
EPS = 1e-6
TOKTILE = 512
USE_SILU = os.environ.get("KERNEL_NO_SILU", "") != "1"

HALF = [s // 2 for s in SPAN_GROUPS]
PSZ_R = [h + 2 for h in HALF]
PSZ_C = [s + 1 for s in SPAN_GROUPS]
T_R = [h + 1 for h in HALF]
T_G = [(h + 1) * s for h, s in zip(HALF, SPAN_GROUPS)]
CENTER = [h * s + h for h, s in zip(HALF, SPAN_GROUPS)]


# --------------------------------------------------------------------------
# host-side routing / packing
# --------------------------------------------------------------------------

def _routing(inputs):
    pairs = [[[] for _ in range(G)] for _ in range(N_CORES)]
    for s_id in range(2):
        flow = np.asarray(inputs[f"flow_map_{s_id}"])[0]
        sx = np.asarray(inputs[f"adaptive_spans_x_{s_id}"])[0]
        sy = np.asarray(inputs[f"adaptive_spans_y_{s_id}"])[0]
        yy, xx = np.meshgrid(np.arange(H, dtype=np.float32),
                             np.arange(W, dtype=np.float32), indexing="ij")
        cy = np.clip(yy + flow[..., 1], 0, H - 1).astype(np.float32)
        cx = np.clip(xx + flow[..., 0], 0, W - 1).astype(np.float32)
        y0i = np.floor(cy).astype(np.int64)
        x0i = np.floor(cx).astype(np.int64)
        fy = (cy - y0i).astype(np.float32)
        fx = (cx - x0i).astype(np.float32)
        avg = ((sx + sy) * 0.5).astype(np.float32)
        for n in range(H * W):
            yi, xi = divmod(n, W)
            p_glob = s_id * H * W + n
            core, local = p_glob % N_CORES, p_glob // N_CORES
            for g, s in enumerate(SPAN_GROUPS):
                if abs(float(avg[yi, xi]) - s) < TOL:
                    pairs[core][g].append(
                        (local, s_id, int(y0i[yi, xi]), int(x0i[yi, xi]),
                         float(fy[yi, xi]), float(fx[yi, xi])))
    for core in range(N_CORES):
        for g in range(G):
            pairs[core][g].sort(key=lambda t: (t[1], t[2]))
    return pairs


def _caps_from_pairs(pairs):
    caps = []
    for g in range(G):
        m = max(len(pairs[c][g]) for c in range(N_CORES))
        caps.append(max(2, int(math.ceil(m / 2.0)) * 2))
    return tuple(caps)


def _tile_pixel_sets(pairs_g, cap, g):
    """Per token-tile: sorted list of (stream, pixel) the tile's pairs use."""
    tg, half, pr, pc = T_G[g], HALF[g], PSZ_R[g], PSZ_C[g]
    X = cap * tg
    ntt = (X + TOKTILE - 1) // TOKTILE
    sets = [set() for _ in range(ntt)]
    for i, (local, s_id, y0, x0, fy, fx) in enumerate(pairs_g):
        r0, c0 = y0 - half + PAD, x0 - half + PAD
        pix = [(s_id, (r0 + r) * IMG + c0 + c)
               for r in range(pr) for c in range(pc)]
        for tt in range(i * tg // TOKTILE, ((i + 1) * tg - 1) // TOKTILE + 1):
            if tt < ntt:
                sets[tt].update(pix)
    return [sorted(ps) for ps in sets]


def _nslots(pairs, caps):
    """Per-group, per-token-tile slot counts (max across cores)."""
    ns = []
    for g in range(G):
        X = caps[g] * T_G[g]
        ntt = (X + TOKTILE - 1) // TOKTILE
        per_tt = [1] * ntt
        for core in range(N_CORES):
            for tt, ps in enumerate(
                    _tile_pixel_sets(pairs[core][g], caps[g], g)):
                per_tt[tt] = max(per_tt[tt], (len(ps) + 127) // 128)
        ns.append(tuple(per_tt))
    return tuple(ns)


def _pack_core(inputs, pairs_core, caps, nslots):
    imgs = []
    for s_id in range(2):
        feat = np.asarray(inputs[f"feat_match_{s_id}"])[0]
        img = np.zeros((C, IMG, IMG), np.float32)
        img[:, PAD:PAD + H, PAD:PAD + W] = feat
        imgs.append(img.reshape(C, NPIX))
    out = {}
    for g, s in enumerate(SPAN_GROUPS):
        cap, half, tg = caps[g], HALF[g], T_G[g]
        nsl_tt = nslots[g]
        X = cap * tg
        ntt = (X + TOKTILE - 1) // TOKTILE
        soff = np.cumsum([0] + list(nsl_tt))  # slot offsets per tile
        tot_sl = int(soff[-1])
        psets = _tile_pixel_sets(pairs_core[g], cap, g)
        pidx = [{p: j for j, p in enumerate(ps)} for ps in psets]
        bands = np.zeros((128, tot_sl, 2, 128), np.float32)
        spack = np.zeros((128, tot_sl, TOKTILE), np.float32)
        zones = np.ones((128, cap, tg), np.float32)
        zones[:, :, 0] = 0.0
        E = np.zeros((cap + (1 if g == 0 else 0), P_CORE), np.float32)
        for tt in range(ntt):
            for j, (s_id, pix) in enumerate(psets[tt]):
                sl, row = divmod(j, 128)
                bands[row, soff[tt] + sl, :, :] = \
                    imgs[s_id][:, pix].reshape(2, 128)
        for i, (local, s_id, y0, x0, fy, fx) in enumerate(pairs_core[g]):
            E[i, local] = 1.0
            r0, c0 = y0 - half + PAD, x0 - half + PAD
            ws = ((1 - fy) * (1 - fx), (1 - fy) * fx,
                  fy * (1 - fx), fy * fx)
            dydx = ((0, 0), (0, 1), (1, 0), (1, 1))
            for l in range(tg):
                iy, ix = divmod(l, s)
                col = i * tg + l
                tt, colin = divmod(col, TOKTILE)
                for w, (dy, dx) in zip(ws, dydx):
                    pix = (s_id, (r0 + iy + dy) * IMG + c0 + ix + dx)
                    sl, row = divmod(pidx[tt][pix], 128)
                    spack[row, soff[tt] + sl, colin] += w
        if g == 0:
            E[cap, :] = 1.0
        out[f"bands_{g}"] = np.ascontiguousarray(
            bands.reshape(128, -1)).astype(NPBF16)
        out[f"spack_{g}"] = np.ascontiguousarray(
            spack.reshape(128, -1)).astype(NPBF16)
        out[f"zones_{g}"] = np.ascontiguousarray(
            zones.reshape(128, X)).astype(NPBF16)
        out[f"emat_{g}"] = np.ascontiguousarray(E).astype(NPBF16)
    return out


def _pack_weights(inputs):
    Win = np.asarray(inputs["Win"], np.float32)
    alog = np.asarray(inputs["alog"], np.float32)
    Wout = np.asarray(inputs["Wout"], np.float32)
    Wgeom = np.asarray(inputs["Wgeom"], np.float32)
    gamma = np.asarray(inputs["gamma"], np.float32)
    Wagg = np.asarray(inputs["Wagg"], np.float32)
    bagg = np.asarray(inputs["bagg"], np.float32)
    Wagg_g = np.asarray(inputs["Wagg_g"], np.float32)
    bagg_g = np.asarray(inputs["bagg_g"], np.float32)

    Winp = Win * gamma[..., None]
    wi = Winp.reshape(G, 2, 2, 128, 2 * C).transpose(3, 0, 1, 2, 4)
    wo = Wout.reshape(G, 2, 2, 128, C).transpose(3, 0, 1, 2, 4)
    wg = Wgeom[:, 1].reshape(G, 2, 128, DG).transpose(2, 0, 1, 3)
    wa = Wagg.reshape(G, 2, 128, C).transpose(2, 0, 1, 3)
    wag = Wagg_g.reshape(G, DG, DG).transpose(1, 0, 2)
    a = 1.0 / (1.0 + np.exp(-alog))
    av = a.reshape(G, 2, 2, 128).transpose(3, 0, 1, 2)
    return {
        "w_in": np.ascontiguousarray(wi.reshape(128, -1)).astype(NPBF16),
        "w_out": np.ascontiguousarray(wo.reshape(128, -1)).astype(NPBF16),
        "w_geom": np.ascontiguousarray(wg.reshape(128, -1)).astype(NPBF16),
        "w_agg": np.ascontiguousarray(wa.reshape(128, -1)).astype(NPBF16),
        "w_agg_g": np.ascontiguousarray(wag.reshape(DG, -1)).astype(NPBF16),
        "avals": np.ascontiguousarray(av.reshape(128, -1)),
        "bagg_row": bagg.reshape(1, C).astype(NPBF16),
        "bagg_g_row": bagg_g.reshape(1, DG).astype(NPBF16),
        "eye": np.eye(128, dtype=NPBF16),
    }


# --------------------------------------------------------------------------
# device program
# --------------------------------------------------------------------------

def build_program(caps, nslots):
    nc = bacc.Bacc("TRN2", target_bir_lowering=False, debug=False,
                   num_devices=N_CORES)
    din = {}

    def inp(name, shape, dt=BF16):
        din[name] = nc.dram_tensor(name, shape, dt,
                                   kind="ExternalInput").ap()

    inp("w_in", [128, G * 2 * 2 * 2 * C])
    inp("w_out", [128, G * 2 * 2 * C])
    inp("w_geom", [128, G * 2 * DG])
    inp("w_agg", [128, G * 2 * C])
    inp("w_agg_g", [DG, G * DG])
    inp("avals", [128, G * 2 * 2], F32)
    inp("bagg_row", [1, C])
    inp("bagg_g_row", [1, DG])
    inp("eye", [128, 128])
    for g in range(G):
        cap = caps[g]
        X = cap * T_G[g]
        tot_sl = sum(nslots[g])
        inp(f"bands_{g}", [128, tot_sl * 2 * 128])
        inp(f"spack_{g}", [128, tot_sl * TOKTILE])
        inp(f"zones_{g}", [128, X])
        inp(f"emat_{g}", [cap + (1 if g == 0 else 0), P_CORE])

    m_out = nc.dram_tensor("m_slice", [P_CORE, C], F32,
                           kind="ExternalOutput").ap()
    g_out = nc.dram_tensor("g_slice", [P_CORE, DG], F32,
                           kind="ExternalOutput").ap()

    with tile.TileContext(nc) as tc:
        _emit(tc, nc, din, m_out, g_out, caps, nslots)
    nc.compile()
    return nc


def _emit(tc, nc, din, m_out, g_out, caps, nslots):
    from contextlib import ExitStack
    ctx = ExitStack()
    cpool = ctx.enter_context(tc.tile_pool(name="const", bufs=1))
    gpool = ctx.enter_context(tc.tile_pool(name="grp", bufs=1))
    ppool = ctx.enter_context(tc.tile_pool(name="stage", bufs=1))
    mm_ps = ctx.enter_context(tc.tile_pool(name="mmps", bufs=1, space="PSUM"))
    row_ps = ctx.enter_context(tc.tile_pool(name="rowps", bufs=1, space="PSUM"))
    sm_ps = ctx.enter_context(tc.tile_pool(name="smps", bufs=1, space="PSUM"))

    w_in = cpool.tile([128, G * 2 * 2 * 2 * C], BF16)
    nc.sync.dma_start(out=w_in[:], in_=din["w_in"][:])
    w_outw = cpool.tile([128, G * 2 * 2 * C], BF16)
    nc.sync.dma_start(out=w_outw[:], in_=din["w_out"][:])
    w_geom = cpool.tile([128, G * 2 * DG], BF16)
    nc.sync.dma_start(out=w_geom[:], in_=din["w_geom"][:])
    w_agg = cpool.tile([128, G * 2 * C], BF16)
    nc.sync.dma_start(out=w_agg[:], in_=din["w_agg"][:])
    w_agg_g = cpool.tile([DG, G * DG], BF16)
    nc.sync.dma_start(out=w_agg_g[:], in_=din["w_agg_g"][:])
    avals = cpool.tile([128, G * 2 * 2], F32)
    nc.sync.dma_start(out=avals[:], in_=din["avals"][:])
    eye = cpool.tile([128, 128], BF16)
    nc.sync.dma_start(out=eye[:], in_=din["eye"][:])
    ones_col = cpool.tile([128, 1], BF16)
    nc.vector.memset(ones_col[:], 1.0)
    eps_tile = cpool.tile([128, 1], F32)
    nc.vector.memset(eps_tile[:], EPS)
    ones_row = cpool.tile([1, 128], BF16)
    nc.vector.memset(ones_row[:], 1.0)
    rs_ones_wide = cpool.tile([1, TOKTILE], BF16)
    nc.vector.memset(rs_ones_wide[:], 1.0)

    def win_lhsT(g, d, k, m):
        base = ((g * 2 + d) * 2 + k) * (2 * C)
        return w_in[:, base + m * 128: base + (m + 1) * 128]

    def wout_lhsT(g, d, k, m):
        base = ((g * 2 + d) * 2 + k) * C
        return w_outw[:, base + m * 128: base + (m + 1) * 128]

    def wgeom_lhsT(g, k):
        return w_geom[:, (g * 2 + k) * DG: (g * 2 + k + 1) * DG]

    def wagg_lhsT(g, k, m):
        base = (g * 2 + k) * C
        return w_agg[:, base + m * 128: base + (m + 1) * 128]

    m_acc = cpool.tile([P_CORE, C], F32)
    g_acc = cpool.tile([P_CORE, DG], F32)


    for g in range(G):
        s = SPAN_GROUPS[g]
        cap, half = caps[g], HALF[g]
        tg, cen, nsl_tt = T_G[g], CENTER[g], nslots[g]
        X = cap * tg
        ntt = (X + TOKTILE - 1) // TOKTILE
        soff = [0]
        for v in nsl_tt:
            soff.append(soff[-1] + v)
        tot_sl = soff[-1]
        capg = cap + (1 if g == 0 else 0)

        bands = ppool.tile([128, tot_sl * 2 * 128], BF16, tag="bands")
        nc.sync.dma_start(out=bands[:], in_=din[f"bands_{g}"][:])
        spack = ppool.tile([128, tot_sl * TOKTILE], BF16, tag="spack")
        nc.sync.dma_start(out=spack[:], in_=din[f"spack_{g}"][:])
        zones = ppool.tile([128, X], BF16, tag="zones")
        nc.sync.dma_start(out=zones[:], in_=din[f"zones_{g}"][:])
        emat = ppool.tile([capg, P_CORE], BF16, tag="emat")
        nc.sync.dma_start(out=emat[:], in_=din[f"emat_{g}"][:])

        def bands_lhsT(tt, sl, m):
            base = ((soff[tt] + sl) * 2 + m) * 128
            return bands[:, base: base + 128]

        def spack_rhs(tt, sl, tl):
            base = (soff[tt] + sl) * TOKTILE
            return spack[:, base: base + tl]

        x0 = gpool.tile([128, 2, X], BF16, tag="x0", bufs=2)
        x1 = gpool.tile([128, 2, X], BF16, tag="x1")
        u_t = gpool.tile([128, 2, X], BF16, tag="uz")
        z_t = u_t  # sigmoid scratch in d1; u in d2 (disjoint lifetimes)
        h_t = gpool.tile([128, 2, X], BF16, tag="h")
        y_t = gpool.tile([128, 2, X], BF16, tag="y")
        x2 = gpool.tile([128, 2, X], BF16, tag="x2", bufs=2)
        xn = gpool.tile([128, 2, X], BF16, tag="xn")
        a_t = gpool.tile([128, 2, X], F32, tag="a_t")
        s_row = gpool.tile([1, X], F32, tag="s_row")
        r_row = gpool.tile([1, X], F32, tag="r_row")
        rs_row = gpool.tile([1, X], BF16, tag="rs_row")
        rs_sb = gpool.tile([128, X], BF16, tag="rs_sb")

        # ---- windows via PE: win = bands^T @ S; extract x0 + squares
        for tt in range(ntt):
            t0 = tt * TOKTILE
            tl = min(TOKTILE, X - t0)
            for m in range(2):
                wp = mm_ps.tile([128, TOKTILE], F32, tag=("w0", "w1")[m])
                for sl in range(nsl_tt[tt]):
                    nc.tensor.matmul(wp[:, :tl], bands_lhsT(tt, sl, m),
                                     spack_rhs(tt, sl, tl),
                                     start=(sl == 0),
                                     stop=(sl == nsl_tt[tt] - 1))
                nc.scalar.activation(out=x0[:, m, t0:t0 + tl],
                                     in_=wp[:, :tl], func=ACTF.Copy)
                nc.scalar.activation(out=x2[:, m, t0:t0 + tl],
                                     in_=wp[:, :tl], func=ACTF.Square)

        for d in range(DEPTH):
            if d == 1:
                for ch in range(2):
                    nc.scalar.activation(out=x2[:, ch], in_=x1[:, ch],
                                         func=ACTF.Square)
            # rs row: 1/sqrt(mean(x^2) + eps)
            for t0 in range(0, X, TOKTILE):
                tl = min(TOKTILE, X - t0)
                s_ps = row_ps.tile([1, TOKTILE], F32, tag="row")
                for ch in range(2):
                    nc.tensor.matmul(s_ps[:, :tl], ones_col[:],
                                     x2[:, ch, t0:t0 + tl],
                                     start=(ch == 0), stop=(ch == 1))
                nc.scalar.activation(out=s_row[:, t0:t0 + tl],
                                     in_=s_ps[:, :tl], func=ACTF.Copy)
            # rs = exp(-0.5*ln(sum/C + eps)); single Ln/Exp per depth to
            # minimize ACT table switches
            nc.scalar.activation(out=r_row[:], in_=s_row[:], func=ACTF.Ln,
                                 scale=1.0 / C, bias=eps_tile[:1, :])
            nc.scalar.activation(out=rs_row[:], in_=r_row[:],
                                 func=ACTF.Exp, scale=-0.5)
            for t0 in range(0, X, TOKTILE):
                tl = min(TOKTILE, X - t0)
                bc_ps = row_ps.tile([128, TOKTILE], F32, tag="row")
                nc.tensor.matmul(bc_ps[:, :tl], ones_row[:],
                                 rs_row[:, t0:t0 + tl], start=True, stop=True)
                nc.scalar.activation(out=rs_sb[:, t0:t0 + tl],
                                     in_=bc_ps[:, :tl], func=ACTF.Copy)

            # decay tile (fp32): zones * a  (DVE TS, per token-tile)
            for ch in range(2):
                av = avals[:, (g * 2 + d) * 2 + ch: (g * 2 + d) * 2 + ch + 1]
                for t0 in range(0, X, TOKTILE):
                    tl = min(TOKTILE, X - t0)
                    nc.vector.tensor_scalar_mul(
                        out=a_t[:, ch, t0:t0 + tl],
                        in0=zones[:, t0:t0 + tl], scalar1=av)

            if d == 0:
                for tt0 in range(ntt):
                    t0 = tt0 * TOKTILE
                    tl = min(TOKTILE, X - t0)
                    for ch in range(2):
                        nc.vector.tensor_tensor(
                            out=xn[:, ch, t0:t0 + tl],
                            in0=x0[:, ch, t0:t0 + tl],
                            in1=rs_sb[:, t0:t0 + tl], op=ALU.mult)
                for tt in range(ntt):
                    t0 = tt * TOKTILE
                    tl = min(TOKTILE, X - t0)
                    uzps = []
                    for m in range(4):
                        ps = mm_ps.tile([128, TOKTILE], F32,
                                        tag=("u0", "u1", "z0", "z1")[m])
                        for k in range(2):
                            nc.tensor.matmul(ps[:, :tl], win_lhsT(g, d, k, m),
                                             xn[:, k, t0:t0 + tl],
                                             start=(k == 0), stop=(k == 1))
                        uzps.append(ps)
                    for ch in range(2):
                        init = 0.0 if tt == 0 else h_t[:, ch, t0 - 1:t0]
                        nc.vector.tensor_tensor_scan(
                            out=h_t[:, ch, t0:t0 + tl],
                            data0=a_t[:, ch, t0:t0 + tl],
                            data1=uzps[ch][:, :tl],
                            initial=init, op0=ALU.mult, op1=ALU.add)
                        if USE_SILU:
                            nc.scalar.activation(
                                out=z_t[:, ch, t0:t0 + tl],
                                in_=uzps[2 + ch][:, :tl], func=ACTF.Silu)
                            nc.vector.tensor_tensor(
                                out=y_t[:, ch, t0:t0 + tl],
                                in0=h_t[:, ch, t0:t0 + tl],
                                in1=z_t[:, ch, t0:t0 + tl], op=ALU.mult)
                        else:
                            nc.scalar.activation(
                                out=z_t[:, ch, t0:t0 + tl],
                                in_=uzps[2 + ch][:, :tl], func=ACTF.Sigmoid)
                            nc.vector.tensor_tensor(
                                out=y_t[:, ch, t0:t0 + tl],
                                in0=h_t[:, ch, t0:t0 + tl],
                                in1=z_t[:, ch, t0:t0 + tl], op=ALU.mult)
                            nc.vector.tensor_tensor(
                                out=y_t[:, ch, t0:t0 + tl],
                                in0=y_t[:, ch, t0:t0 + tl],
                                in1=uzps[2 + ch][:, :tl], op=ALU.mult)
                for tt in range(ntt):
                    t0 = tt * TOKTILE
                    tl = min(TOKTILE, X - t0)
                    for m in range(2):
                        ps = mm_ps.tile([128, TOKTILE], F32,
                                        tag=("u0", "u1")[m])
                        for k in range(2):
                            nc.tensor.matmul(ps[:, :tl], wout_lhsT(g, d, k, m),
                                             y_t[:, k, t0:t0 + tl],
                                             start=(k == 0), stop=(k == 1))
                        nc.vector.tensor_tensor(
                            out=x1[:, m, t0:t0 + tl], in0=ps[:, :tl],
                            in1=x0[:, m, t0:t0 + tl], op=ALU.add)
            else:
                for tt in range(ntt):
                    t0 = tt * TOKTILE
                    tl = min(TOKTILE, X - t0)
                    for ch in range(2):
                        nc.vector.tensor_tensor(
                            out=xn[:, ch, t0:t0 + tl],
                            in0=x1[:, ch, t0:t0 + tl],
                            in1=rs_sb[:, t0:t0 + tl], op=ALU.mult)
                    for m in range(2):
                        ps = mm_ps.tile([128, TOKTILE], F32,
                                        tag=("u0", "u1")[m])
                        for k in range(2):
                            nc.tensor.matmul(ps[:, :tl], win_lhsT(g, d, k, m),
                                             xn[:, k, t0:t0 + tl],
                                             start=(k == 0), stop=(k == 1))
                        init = 0.0 if tt == 0 else h_t[:, m, t0 - 1:t0]
                        nc.vector.tensor_tensor_scan(
                            out=h_t[:, m, t0:t0 + tl],
                            data0=a_t[:, m, t0:t0 + tl],
                            data1=ps[:, :tl],
                            initial=init, op0=ALU.mult, op1=ALU.add)

        # ---- depth-2 center-only tail
        h_c = [h_t[:, ch].rearrange("p (a b) -> p a b", a=cap)[:, :, cen]
               for ch in range(2)]
        x1_c = [x1[:, ch].rearrange("p (a b) -> p a b", a=cap)[:, :, cen]
                for ch in range(2)]
        xn_c = [xn[:, ch].rearrange("p (a b) -> p a b", a=cap)[:, :, cen]
                for ch in range(2)]

        z_c = gpool.tile([128, 2, cap], BF16, tag="z_c")
        y_c = gpool.tile([128, 2, cap], BF16, tag="y_c")
        mc = gpool.tile([128, 2, cap], BF16, tag="mc")
        gc = gpool.tile([DG, cap], BF16, tag="gc")

        for m in range(2):
            ps = sm_ps.tile([128, cap], F32, tag="smps")
            for k in range(2):
                nc.tensor.matmul(ps[:], win_lhsT(g, 1, k, 2 + m),
                                 xn_c[k], start=(k == 0), stop=(k == 1))
            nc.scalar.activation(out=z_c[:, m], in_=ps[:], func=ACTF.Copy)
        for ch in range(2):
            if USE_SILU:
                nc.scalar.activation(out=z_c[:, ch], in_=z_c[:, ch],
                                     func=ACTF.Silu)
                nc.vector.tensor_tensor(out=y_c[:, ch], in0=h_c[ch],
                                        in1=z_c[:, ch], op=ALU.mult)
            else:
                nc.vector.tensor_tensor(out=y_c[:, ch], in0=h_c[ch],
                                        in1=z_c[:, ch], op=ALU.mult)
                nc.scalar.activation(out=z_c[:, ch], in_=z_c[:, ch],
                                     func=ACTF.Sigmoid)
                nc.vector.tensor_tensor(out=y_c[:, ch], in0=y_c[:, ch],
                                        in1=z_c[:, ch], op=ALU.mult)
        for m in range(2):
            ps = sm_ps.tile([128, cap], F32, tag="smps")
            for k in range(2):
                nc.tensor.matmul(ps[:], wout_lhsT(g, 1, k, m),
                                 y_c[:, k], start=(k == 0), stop=(k == 1))
            nc.vector.tensor_tensor(out=mc[:, m], in0=ps[:], in1=x1_c[m],
                                    op=ALU.add)
        ps = row_ps.tile([DG, cap], F32, tag="row")
        for k in range(2):
            nc.tensor.matmul(ps[:], wgeom_lhsT(g, k), y_c[:, k],
                             start=(k == 0), stop=(k == 1))
        nc.scalar.activation(out=gc[:], in_=ps[:], func=ACTF.Copy)

        # ---- aggregation
        pcon = gpool.tile([128, 2, cap], BF16, tag="pcon")
        for m in range(2):
            ps = sm_ps.tile([128, cap], F32, tag="smps")
            for k in range(2):
                nc.tensor.matmul(ps[:], wagg_lhsT(g, k, m), mc[:, k],
                                 start=(k == 0), stop=(k == 1))
            nc.scalar.activation(out=pcon[:, m], in_=ps[:], func=ACTF.Copy)
        gcon = gpool.tile([DG, cap], BF16, tag="gcon")
        ps = row_ps.tile([DG, cap], F32, tag="row")
        nc.tensor.matmul(ps[:], w_agg_g[:, g * DG:(g + 1) * DG], gc[:],
                         start=True, stop=True)
        nc.scalar.activation(out=gcon[:], in_=ps[:], func=ACTF.Copy)

        pairT = gpool.tile([capg, C], BF16, tag="pairT")
        gpairT = gpool.tile([capg, DG], BF16, tag="gpairT")
        for m in range(2):
            tp = sm_ps.tile([cap, 128], BF16, tag="smps")
            nc.tensor.transpose(tp[:], pcon[:, m], eye[:])
            nc.vector.tensor_copy(pairT[:cap, m * 128:(m + 1) * 128], tp[:])
        tpg = row_ps.tile([cap, DG], BF16, tag="row")
        nc.tensor.transpose(tpg[:], gcon[:], eye[:DG, :DG])
        nc.vector.tensor_copy(gpairT[:cap, :], tpg[:])
        if g == 0:
            nc.sync.dma_start(out=pairT[cap:cap + 1, :],
                              in_=din["bagg_row"][:])
            nc.sync.dma_start(out=gpairT[cap:cap + 1, :],
                              in_=din["bagg_g_row"][:])

        mg_ps = sm_ps.tile([P_CORE, C], F32, tag="smps")
        nc.tensor.matmul(mg_ps[:], emat[:], pairT[:], start=True, stop=True)
        gg_ps = row_ps.tile([P_CORE, DG], F32, tag="row")
        nc.tensor.matmul(gg_ps[:], emat[:], gpairT[:], start=True, stop=True)
        if g == 0:
            nc.scalar.activation(out=m_acc[:], in_=mg_ps[:], func=ACTF.Copy)
            nc.scalar.activation(out=g_acc[:], in_=gg_ps[:], func=ACTF.Copy)
        else:
            nc.vector.tensor_tensor(out=m_acc[:], in0=m_acc[:], in1=mg_ps[:],
                                    op=ALU.add)
            nc.vector.tensor_tensor(out=g_acc[:], in0=g_acc[:], in1=gg_ps[:],
                                    op=ALU.add)

    nc.sync.dma_start(out=m_out[:], in_=m_acc[:])
    nc.sync.dma_start(out=g_out[:], in_=g_acc[:])
    ctx.close()


# --------------------------------------------------------------------------
# entry point
# --------------------------------------------------------------------------

_PROGRAM_CACHE = {}


def _get_program(caps, nslots):
    key = (caps, nslots, USE_SILU)
    if key not in _PROGRAM_CACHE:
        _PROGRAM_CACHE[key] = build_program(caps, nslots)
    return _PROGRAM_CACHE[key]


def make_in_maps(inputs):
    pairs = _routing(inputs)
    caps = _caps_from_pairs(pairs)
    nslots = _nslots(pairs, caps)
    wmaps = _pack_weights(inputs)
    in_maps = []
    for core in range(N_CORES):
        m = dict(wmaps)
        m.update(_pack_core(inputs, pairs[core], caps, nslots))
        in_maps.append(m)
    return caps, nslots, in_maps


def _assemble(results):
    m_full = np.zeros((512, C), np.float32)
    g_full = np.zeros((512, DG), np.float32)
    for c in range(N_CORES):
        m_full[c::N_CORES] = results[c]["m_slice"]
        g_full[c::N_CORES] = results[c]["g_slice"]
    outs = []
    for s_id in range(2):
        m = m_full[s_id * 256:(s_id + 1) * 256]
        outs.append(np.ascontiguousarray(
            m.reshape(H, W, C).transpose(2, 0, 1)[None]).astype(np.float32))
    for s_id in range(2):
        gsl = g_full[s_id * 256:(s_id + 1) * 256]
        outs.append(np.ascontiguousarray(
            gsl.reshape(H, W, DG).transpose(2, 0, 1)[None]).astype(np.float32))
    return tuple(outs)


def kernel(**inputs):
    caps, nslots, in_maps = make_in_maps(inputs)
    nc = _get_program(caps, nslots)
    res = run_bass_kernel_spmd(nc, in_maps, list(range(N_CORES)))
    return _assemble(res.results)


if __name__ == "__main__":
    nc = build_program((8, 20, 32, 32, 12), (2, 3, 3, 3, 4))
    print("built+compiled ok")
